# revision 22
# baseline (speedup 1.0000x reference)
"""Trainium2 Bass kernel for nn_AttentionNet (NTS-Net style NMS detection).

Sharding: pure data parallel — batch 8 -> 1 image per NeuronCore (8 cores).
Everything for one image (backbone, proposal net, hard-NMS, crop-resize,
part backbone, heads) runs in a single Bass/Tile kernel on its core.

Key device-side tricks:
  * backbone conv (32x32 stride 32) == patch-embed matmul over im2col APs
  * hard-NMS: vector.max/max_index two-level argmax over a [128,13] score
    layout; anchor coords gathered by one-hot masked reduction; IoU row
    computed in-place with tensor_scalar ops (inter <= TH*union form)
  * crop-resize (align_corners bilinear) == two matmuls with triangular
    interpolation matrices relu(1 - |t - c|) built on device from the box
  * cropped parts bounce through DRAM to re-layout into im2col so the part
    backbone reuses the same resident W_bb and matmul loop
"""

import numpy as np
import ml_dtypes

bf16 = ml_dtypes.bfloat16

TOP_N = 4
PART = 224
IMG = 448
NUM_CLASS = 200
BATCH = 8
N_ANCH = 1614
NP_ANCH = 1664  # padded to 128*13
NEG = -1e30


# ---------------------------------------------------------------- host consts
def _make_edge_anchors():
    aspects = [0.667, 1.0, 1.5]
    settings = [
        (32, 14, 48.0, [2 ** (1.0 / 3), 2 ** (2.0 / 3)]),
        (64, 7, 96.0, [2 ** (1.0 / 3), 2 ** (2.0 / 3)]),
        (128, 4, 192.0, [1.0, 2 ** (1.0 / 3), 2 ** (2.0 / 3)]),
    ]
    boxes = []
    for stride, fm, size, scales in settings:
        for s in scales:
            for ar in aspects:
                h = size * s / np.sqrt(ar)
                w = size * s * np.sqrt(ar)
                c = (np.arange(fm) + 0.5) * stride
                cy, cx = np.meshgrid(c, c, indexing="ij")
                b = np.stack([cy - h / 2, cx - w / 2, cy + h / 2, cx + w / 2], -1)
                boxes.append(b.reshape(-1, 4))
    e = np.clip(np.concatenate(boxes, 0), 0, IMG)
    return e.astype(np.int64)  # UNPADDED coords [0,448]


EDGE_U = _make_edge_anchors()  # [1614,4] int


def _host_consts():
    # cst_f32 [128, 96]
    cst = np.zeros((128, 96), np.float32)
    p = np.arange(128)
    j = np.arange(13)
    cst[:, 0:13] = (p[:, None] * 13 + j[None, :]).astype(np.float32)  # iota2d
    A = EDGE_U.astype(np.float32)
    coords = np.zeros((NP_ANCH, 5), np.float32)
    coords[:N_ANCH, 0:4] = A
    coords[:N_ANCH, 4] = (A[:, 2] - A[:, 0]) * (A[:, 3] - A[:, 1])
    c2 = coords.reshape(128, 13, 5)
    for k in range(5):
        cst[:, 13 + 13 * k:13 + 13 * (k + 1)] = c2[:, :, k]
    cst[:, 78] = p.astype(np.float32)          # iota_p
    cst[:, 79] = 1.0                            # ones col
    for yc in range(4):
        cst[:, 80 + yc] = (p + 112 * yc).astype(np.float32)  # iota_pc
    # cst_row [1, 384]
    cr = np.zeros((1, 512), np.float32)
    cr[0, 0:224] = np.arange(224, dtype=np.float32)
    cr[0, 224:352] = np.arange(128, dtype=np.float32)
    cr[0, 384:512] = 1.0
    ident = np.eye(128, dtype=np.float32)
    return cst, cr, ident


def _prep_weights(inputs):
    """Host-side weight re-layouts (all bf16 except biases)."""
    W = {}
    W["wbb"] = np.ascontiguousarray(
        inputs["W_bb"].reshape(2048, 3 * 32 * 32).T).astype(bf16)  # [3072,2048]
    # W_d1 [128,2048,3,3] -> [16,3,3,128,128] (kc,ky,kx,p_in,m_out)
    wd1 = inputs["W_d1"].reshape(128, 16, 128, 3, 3)
    W["wd1"] = np.ascontiguousarray(wd1.transpose(1, 3, 4, 2, 0)).astype(bf16)
    for nm, key in (("wd2", "W_d2"), ("wd3", "W_d3")):
        w = inputs[key]  # [128,128,3,3]
        W[nm] = np.ascontiguousarray(w.transpose(2, 3, 1, 0)).astype(bf16)  # [3,3,128,128]
    W["wt1"] = np.ascontiguousarray(inputs["W_t1"][:, :, 0, 0].T).astype(bf16)  # [128,6]
    W["wt2"] = np.ascontiguousarray(inputs["W_t2"][:, :, 0, 0].T).astype(bf16)
    W["wt3"] = np.ascontiguousarray(inputs["W_t3"][:, :, 0, 0].T).astype(bf16)  # [128,9]
    # fold mean-pool scales into head weights
    W["wfc"] = np.ascontiguousarray((inputs["W_fc"] / 196.0).T).astype(bf16)  # [2048,200]
    W["wpart"] = np.ascontiguousarray((inputs["W_part"] / 49.0).T).astype(bf16)
    wcat = inputs["W_cat"].copy()  # [200, 10240]
    wcat[:, :8192] /= 49.0
    wcat[:, 8192:] /= 196.0
    W["wcat"] = np.ascontiguousarray(wcat.T).astype(bf16)  # [10240,200]
    bts = np.zeros((96, 1), np.float32)
    bts[0:6, 0] = inputs["b_t1"]
    bts[32:38, 0] = inputs["b_t2"]
    bts[64:73, 0] = inputs["b_t3"]
    W["bts"] = bts
    return W


# ---------------------------------------------------------------- bass kernel
def build_kernel():
    import concourse.bass as bass
    import concourse.mybir as mybir
    from concourse import bacc, tile

    f32 = mybir.dt.float32
    bfl = mybir.dt.bfloat16
    u32 = mybir.dt.uint32
    u8 = mybir.dt.uint8
    Alu = mybir.AluOpType
    Act = mybir.ActivationFunctionType
    AX = mybir.AxisListType

    nc = bacc.Bacc(None, target_bir_lowering=False)

    # ---- I/O ----
    x_in = nc.declare_dram_parameter("x", [3, 448, 448], bfl, isOutput=False)
    p_xim = nc.declare_dram_parameter("xim", [3072, 196], bfl, isOutput=False)
    p_wbb = nc.declare_dram_parameter("wbb", [3072, 2048], bfl, isOutput=False)
    p_wd1 = nc.declare_dram_parameter("wd1", [16, 3, 3, 128, 128], bfl, isOutput=False)
    p_wd2 = nc.declare_dram_parameter("wd2", [3, 3, 128, 128], bfl, isOutput=False)
    p_wd3 = nc.declare_dram_parameter("wd3", [3, 3, 128, 128], bfl, isOutput=False)
    p_wt1 = nc.declare_dram_parameter("wt1", [128, 6], bfl, isOutput=False)
    p_wt2 = nc.declare_dram_parameter("wt2", [128, 6], bfl, isOutput=False)
    p_wt3 = nc.declare_dram_parameter("wt3", [128, 9], bfl, isOutput=False)
    p_wfc = nc.declare_dram_parameter("wfc", [2048, 200], bfl, isOutput=False)
    p_wpart = nc.declare_dram_parameter("wpart", [2048, 200], bfl, isOutput=False)
    p_wcat = nc.declare_dram_parameter("wcat", [10240, 200], bfl, isOutput=False)
    p_bts = nc.declare_dram_parameter("bts", [96, 1], f32, isOutput=False)
    p_cst = nc.declare_dram_parameter("cst", [128, 96], f32, isOutput=False)
    p_crow = nc.declare_dram_parameter("crow", [1, 512], f32, isOutput=False)
    p_ident = nc.declare_dram_parameter("ident", [128, 128], f32, isOutput=False)

    o_raw = nc.declare_dram_parameter("o_raw", [1, 200], f32, isOutput=True)
    o_cat = nc.declare_dram_parameter("o_cat", [1, 200], f32, isOutput=True)
    o_plog = nc.declare_dram_parameter("o_plog", [4, 200], f32, isOutput=True)
    o_tidx = nc.declare_dram_parameter("o_tidx", [1, 4], f32, isOutput=True)
    o_tprob = nc.declare_dram_parameter("o_tprob", [1, 4], f32, isOutput=True)

    with tile.TileContext(nc) as tc:
        with (
            tc.tile_pool(name="res", bufs=1) as res,           # resident
            tc.tile_pool(name="wstr", bufs=2) as wstr,
            tc.tile_pool(name="whead", bufs=8) as whead,         # streamed weights
            tc.tile_pool(name="work", bufs=3) as work,         # working tiles
            tc.tile_pool(name="nms", bufs=1) as nmsp,          # nms state
            tc.tile_pool(name="ps_big", bufs=4, space="PSUM") as ps_big,
            tc.tile_pool(name="ps_sml", bufs=2, space="PSUM") as ps_sml,
            tc.tile_pool(name="dram", bufs=1, space="DRAM") as dpool,
        ):
            DMA = nc.sync.dma_start

            # ---------------- resident loads ----------------
            cst = res.tile([128, 96], f32)
            DMA(cst[:], p_cst[:])
            crow = res.tile([1, 512], f32)
            DMA(crow[:], p_crow[:])
            ident = res.tile([128, 128], f32)
            DMA(ident[:], p_ident[:])
            bts = res.tile([96, 1], f32)
            DMA(bts[:], p_bts[:])

            iota2d = cst[:, 0:13]
            y0a = cst[:, 13:26]
            x0a = cst[:, 26:39]
            y1a = cst[:, 39:52]
            x1a = cst[:, 52:65]
            area_a = cst[:, 65:78]
            ones_col = cst[:, 79:80]
            iota224 = crow[:, 0:224]
            iota128r = crow[:, 224:352]
            ones_row = crow[:, 384:512]

            im2col = res.tile([128, 24, 196], bfl, tag="imcol")
            DMA(im2col[:], p_xim.rearrange("(kc p) n -> p kc n", p=128))
            wbb_src = p_wbb.rearrange("(kc p) m -> p kc m", p=128)
            wbb_m = []
            for m in range(8):
                wt = res.tile([128, 24, 256], bfl, tag=f"wbb{m}")
                eng = nc.sync if m % 2 == 0 else nc.gpsimd
                eng.dma_start(wt[:], wbb_src[:, :, 256 * m:256 * (m + 1)])
                wbb_m.append(wt)
            wt1 = res.tile([128, 6], bfl)
            DMA(wt1[:], p_wt1[:])
            wt2 = res.tile([128, 6], bfl)
            DMA(wt2[:], p_wt2[:])
            wt3 = res.tile([128, 9], bfl)
            DMA(wt3[:], p_wt3[:])
            wd2 = res.tile([128, 9, 128], bfl)
            DMA(wd2[:], p_wd2.rearrange("ky kx p m -> p (ky kx) m"))
            wd3 = res.tile([128, 9, 128], bfl)
            DMA(wd3[:], p_wd3.rearrange("ky kx p m -> p (ky kx) m"))

            # image, [y-chunks] layout for crop matmuls (lhsT)
            img_yx = res.tile([128, 3, 4, 448], bfl)
            nc.gpsimd.dma_start(img_yx[0:112, :, :, :],
                x_in.rearrange("c (yc p) x -> p c yc x", p=112))

            # ---------------- main backbone conv ----------------
            f_sb = res.tile([128, 16, 196], bfl)
            feat = res.tile([128, 16], f32)
            for m in range(16):
                pm = ps_big.tile([128, 196], f32, tag="big")
                for kc in range(24):
                    nc.tensor.matmul(pm[:], wbb_m[m // 2][:, kc, 128 * (m % 2):128 * (m % 2 + 1)],
                                     im2col[:, kc, :],
                                     start=(kc == 0), stop=(kc == 23))
                nc.scalar.activation(f_sb[:, m, :], pm[:], Act.Relu,
                                     accum_out=feat[:, m:m + 1])

            featb = res.tile([128, 16], bfl)
            nc.vector.tensor_copy(featb[:], feat[:])
            rawps = ps_sml.tile([1, 200], f32, tag="sml")
            for m4 in range(4):
                wc = whead.tile([128, 4, 200], bfl, tag="wcat")
                DMA(wc[:], p_wfc.rearrange("(mc p) n -> p mc n", p=128)
                    [:, 4 * m4:4 * (m4 + 1), :])
                for i in range(4):
                    m = 4 * m4 + i
                    nc.tensor.matmul(rawps[:], featb[:, m:m + 1], wc[:, i, :],
                                     start=(m == 0), stop=(m == 15))
            rawsb = work.tile([1, 200], f32, tag="hdsb")
            nc.vector.tensor_copy(rawsb[:], rawps[:])
            DMA(o_raw[:], rawsb[:])
            # ---------------- proposal net ----------------
            fpad = res.tile([128, 16, 16, 16], bfl)
            nc.vector.memset(fpad[:], 0.0)
            nc.vector.tensor_copy(
                fpad[:, :, 1:15, 1:15],
                f_sb.rearrange("p m (y x) -> p m y x", y=14))
            d1ps = ps_big.tile([128, 196], f32, tag="big")
            for kc in range(16):
                wchunk = wstr.tile([128, 9, 128], bfl, tag="wd1")
                eng = nc.sync if kc % 2 == 0 else nc.gpsimd
                eng.dma_start(wchunk[:], p_wd1[kc].rearrange("ky kx p m -> p (ky kx) m"))
                for t9 in range(9):
                    ky, kx = divmod(t9, 3)
                    nc.tensor.matmul(
                        d1ps[:],
                        wchunk[:, t9, :],
                        fpad[:, kc, ky:ky + 14, kx:kx + 14],
                        start=(kc == 0 and t9 == 0), stop=(kc == 15 and t9 == 8))
            d1pad = res.tile([128, 16, 16], bfl)
            nc.vector.memset(d1pad[:], 0.0)
            nc.scalar.activation(
                d1pad[:, 1:15, 1:15],
                d1ps.rearrange("p (y x) -> p y x", y=14)[:],
                Act.Relu)

            t1ps = ps_sml.tile([6, 196], f32, tag="sml")
            nc.tensor.matmul(t1ps[:], wt1[:],
                             d1pad[:, 1:15, 1:15], start=True, stop=True)
            t1sb = work.tile([6, 196], f32, tag="tsb")
            nc.vector.tensor_scalar_add(t1sb[:], t1ps[:], bts[0:6, :])

            d2ps = ps_sml.tile([128, 49], f32, tag="sml")
            for t9 in range(9):
                ky, kx = divmod(t9, 3)
                nc.tensor.matmul(
                    d2ps[:], wd2[:, t9, :],
                    d1pad[:, ky:ky + 14:2, kx:kx + 14:2],
                    start=(t9 == 0), stop=(t9 == 8))
            d2pad = res.tile([128, 9, 9], bfl)
            nc.vector.memset(d2pad[:], 0.0)
            nc.scalar.activation(d2pad[:, 1:8, 1:8],
                                 d2ps.rearrange("p (y x) -> p y x", y=7)[:],
                                 Act.Relu)
            t2ps = ps_sml.tile([6, 49], f32, tag="sml")
            nc.tensor.matmul(t2ps[:], wt2[:], d2pad[:, 1:8, 1:8],
                             start=True, stop=True)
            t2sb = work.tile([6, 49], f32, tag="tsb")
            nc.vector.tensor_scalar_add(t2sb[:], t2ps[:], bts[32:38, :])

            d3ps = ps_sml.tile([128, 16], f32, tag="sml")
            for t9 in range(9):
                ky, kx = divmod(t9, 3)
                nc.tensor.matmul(
                    d3ps[:], wd3[:, t9, :],
                    d2pad[:, ky:ky + 7:2, kx:kx + 7:2],
                    start=(t9 == 0), stop=(t9 == 8))
            d3sb = work.tile([128, 16], bfl, tag="d3sb")
            nc.scalar.activation(d3sb[:], d3ps[:], Act.Relu)
            t3ps = ps_sml.tile([9, 16], f32, tag="sml")
            nc.tensor.matmul(t3ps[:], wt3[:], d3sb[:], start=True, stop=True)
            t3sb = work.tile([9, 16], f32, tag="tsb")
            nc.vector.tensor_scalar_add(t3sb[:], t3ps[:], bts[64:73, :])

            # assemble scores through DRAM bounce -> [128,13]
            sc_d = dpool.tile([1664], f32)
            DMA(sc_d[0:1176].rearrange("(a n) -> a n", a=6), t1sb[:])
            DMA(sc_d[1176:1470].rearrange("(a n) -> a n", a=6), t2sb[:])
            DMA(sc_d[1470:1614].rearrange("(a n) -> a n", a=9), t3sb[:])
            padt = work.tile([1, 50], f32, tag="padt")
            nc.vector.memset(padt[:], NEG)
            DMA(sc_d[1614:1664].rearrange("(a n) -> a n", a=1), padt[:])
            scores = nmsp.tile([128, 13], f32)
            DMA(scores[:], sc_d.rearrange("(p j) -> p j", p=128))

            # ---------------- NMS (4 rounds) ----------------
            neginf2d = nmsp.tile([128, 13], f32)
            nc.vector.memset(neginf2d[:], NEG)
            topidx = nmsp.tile([1, 4], f32)
            topprob = nmsp.tile([1, 4], f32)
            boxes = nmsp.tile([1, 16], f32)
            scratch = nmsp.tile([128, 13], f32)
            iy_t = nmsp.tile([128, 13], f32)
            ix_t = nmsp.tile([128, 13], f32)
            un_t = nmsp.tile([128, 13], f32)
            supp = nmsp.tile([128, 13], u8)
            csum = nmsp.tile([128, 5], f32)

            for t in range(TOP_N):
                pm8 = nmsp.tile([128, 8], f32, tag="pm8")
                nc.vector.max(pm8[:], scores[:])
                fi8 = nmsp.tile([128, 8], u32, tag="fi8")
                nc.vector.max_index(fi8[:], pm8[:], scores[:])
                fjf = nmsp.tile([128, 1], f32, tag="fjf")
                nc.vector.tensor_copy(fjf[:], fi8[:, 0:1])
                trpsA = ps_sml.tile([1, 128], f32, tag="sml")
                nc.tensor.transpose(trpsA[:], pm8[:, 0:1], ident[:])
                trowA = nmsp.tile([1, 128], f32, tag="trowA")
                nc.vector.tensor_copy(trowA[:], trpsA[:])
                trpsB = ps_sml.tile([1, 128], f32, tag="sml")
                nc.tensor.transpose(trpsB[:], fjf[:], ident[:])
                trowB = nmsp.tile([1, 128], f32, tag="trowB")
                nc.vector.tensor_copy(trowB[:], trpsB[:])
                g8 = nmsp.tile([1, 8], f32, tag="g8")
                nc.vector.max(g8[:], trowA[:])
                nc.vector.tensor_copy(topprob[:, t:t + 1], g8[:, 0:1])
                gi8 = nmsp.tile([1, 8], u32, tag="gi8")
                nc.vector.max_index(gi8[:], g8[:], trowA[:])
                pstar = nmsp.tile([1, 2], f32, tag="pstar")
                nc.vector.tensor_copy(pstar[:, 0:1], gi8[:, 0:1])
                # jstar = jrow[p*]
                mrow = nmsp.tile([1, 128], f32, tag="mrow")
                nc.vector.scalar_tensor_tensor(
                    mrow[:], iota128r, pstar[:, 0:1], trowB[:],
                    Alu.is_equal, Alu.mult, accum_out=pstar[:, 1:2])
                # flat = p*13 + j
                nc.vector.scalar_tensor_tensor(
                    topidx[:, t:t + 1], pstar[:, 0:1], 13.0, pstar[:, 1:2],
                    Alu.mult, Alu.add)
                # broadcast flat to [128,1]
                fb_ps = ps_sml.tile([128, 1], f32, tag="sml")
                nc.tensor.matmul(fb_ps[:], ones_row, topidx[:, t:t + 1],
                                 start=True, stop=True)
                flatb = nmsp.tile([128, 1], f32, tag="flatb")
                nc.vector.tensor_copy(flatb[:], fb_ps[:])
                # gather 5 coords of picked anchor
                for k, cv in enumerate((y0a, x0a, y1a, x1a, area_a)):
                    nc.vector.scalar_tensor_tensor(
                        scratch[:], iota2d, flatb[:], cv,
                        Alu.is_equal, Alu.mult, accum_out=csum[:, k:k + 1])
                s5ps = ps_sml.tile([1, 5], f32, tag="sml")
                nc.tensor.matmul(s5ps[:], ones_col, csum[:], start=True, stop=True)
                s5 = nmsp.tile([1, 5], f32, tag="s5")
                nc.vector.tensor_copy(s5[:], s5ps[:])
                nc.vector.tensor_copy(boxes[:, 4 * t:4 * t + 4], s5[:, 0:4])
                s5b_ps = ps_sml.tile([128, 5], f32, tag="sml")
                nc.tensor.matmul(s5b_ps[:], ones_row, s5[:], start=True, stop=True)
                s5b = nmsp.tile([128, 5], f32, tag="s5b")
                nc.vector.tensor_copy(s5b[:], s5b_ps[:])
                # IoU row and suppression
                nc.vector.tensor_scalar_max(scratch[:], y0a, s5b[:, 0:1])
                nc.vector.scalar_tensor_tensor(
                    iy_t[:], y1a, s5b[:, 2:3], scratch[:], Alu.min, Alu.subtract)
                nc.vector.tensor_scalar_max(iy_t[:], iy_t[:], 0.0)
                nc.vector.tensor_scalar_max(scratch[:], x0a, s5b[:, 1:2])
                nc.vector.scalar_tensor_tensor(
                    ix_t[:], x1a, s5b[:, 3:4], scratch[:], Alu.min, Alu.subtract)
                nc.vector.tensor_scalar_max(ix_t[:], ix_t[:], 0.0)
                nc.vector.tensor_tensor(ix_t[:], iy_t[:], ix_t[:], Alu.mult)  # inter
                nc.vector.scalar_tensor_tensor(
                    un_t[:], area_a, s5b[:, 4:5], ix_t[:], Alu.add, Alu.subtract)
                # suppress where 0.25*union < inter
                nc.vector.scalar_tensor_tensor(
                    supp[:], un_t[:], 0.25, ix_t[:], Alu.mult, Alu.is_lt)
                nc.vector.copy_predicated(scores[:], supp[:], neginf2d[:])

            DMA(o_tidx[:], topidx[:])
            DMA(o_tprob[:], topprob[:])

            # ---------------- crop-resize (4 parts) ----------------
            ty_all = nmsp.tile([1, 896], f32)
            tx_all = nmsp.tile([1, 896], f32)
            sm = nmsp.tile([1, 8], f32)  # per part: sy223, sx223
            for t in range(TOP_N):
                y0b = boxes[:, 4 * t:4 * t + 1]
                x0b = boxes[:, 4 * t + 1:4 * t + 2]
                y1b = boxes[:, 4 * t + 2:4 * t + 3]
                x1b = boxes[:, 4 * t + 3:4 * t + 4]
                nc.vector.scalar_tensor_tensor(
                    sm[:, 2 * t:2 * t + 1], y1b, -1.0, y0b, Alu.add, Alu.subtract)
                nc.vector.tensor_scalar_mul(
                    sm[:, 2 * t:2 * t + 1], sm[:, 2 * t:2 * t + 1], 1.0 / 223.0)
                nc.vector.tensor_scalar(
                    ty_all[:, 224 * t:224 * (t + 1)], iota224,
                    sm[:, 2 * t:2 * t + 1], y0b, Alu.mult, Alu.add)
                nc.vector.scalar_tensor_tensor(
                    sm[:, 2 * t + 1:2 * t + 2], x1b, -1.0, x0b, Alu.add, Alu.subtract)
                nc.vector.tensor_scalar_mul(
                    sm[:, 2 * t + 1:2 * t + 2], sm[:, 2 * t + 1:2 * t + 2], 1.0 / 223.0)
                nc.vector.tensor_scalar(
                    tx_all[:, 224 * t:224 * (t + 1)], iota224,
                    sm[:, 2 * t + 1:2 * t + 2], x0b, Alu.mult, Alu.add)
            tyb = nmsp.tile([128, 2, 448], f32)
            txb = nmsp.tile([128, 2, 448], f32)
            for h in range(2):
                bps = ps_sml.tile([128, 448], f32, tag="sml")
                nc.tensor.matmul(bps[:], ones_row,
                                 ty_all[:, 448 * h:448 * (h + 1)], start=True, stop=True)
                nc.vector.tensor_copy(tyb[:, h, :], bps[:])
                bps2 = ps_sml.tile([128, 448], f32, tag="sml")
                nc.tensor.matmul(bps2[:], ones_row,
                                 tx_all[:, 448 * h:448 * (h + 1)], start=True, stop=True)
                nc.vector.tensor_copy(txb[:, h, :], bps2[:])
            tybf = tyb.rearrange("p h n -> p (h n)")
            txbf = txb.rearrange("p h n -> p (h n)")

            ayt = res.tile([128, 4, 896], bfl)  # [p, yc, t*224+i]
            axt = res.tile([128, 4, 896], bfl)
            wtmp32 = nmsp.tile([128, 896], f32)
            wtmp = nmsp.tile([128, 896], bfl)
            for yc in range(4):
                nc.vector.tensor_scalar_sub(wtmp32[:], tybf[:, :],
                                            cst[:, 80 + yc:81 + yc])
                nc.scalar.activation(wtmp[:], wtmp32[:], Act.Abs)
                nc.scalar.activation(ayt[:, yc, :], wtmp[:], Act.Relu,
                                     bias=ones_col, scale=-1.0)
                nc.vector.tensor_scalar_sub(wtmp32[:], txbf[:, :],
                                            cst[:, 80 + yc:81 + yc])
                nc.scalar.activation(wtmp[:], wtmp32[:], Act.Abs)
                nc.scalar.activation(axt[:, yc, :], wtmp[:], Act.Relu,
                                     bias=ones_col, scale=-1.0)

            # crop matmuls + DRAM bounce into part im2col layout
            S2 = dpool.tile([4, 3, 224, 32, 7], bfl)  # [t, c, i, dx, pj]
            for t in range(TOP_N):
                for c in range(3):
                    T_sb = work.tile([128, 4, 224], bfl, tag="Tsb")
                    for xc in range(4):
                        tp = ps_big.tile([128, 224], f32, tag="big")
                        for yc in range(4):
                            nc.tensor.matmul(
                                tp[0:112, :],
                                img_yx[0:112, c, yc, xc * 112:(xc + 1) * 112],
                                ayt[0:112, yc, 224 * t:224 * (t + 1)],
                                start=(yc == 0), stop=(yc == 3))
                        nc.vector.tensor_copy(T_sb[0:112, xc, :], tp[0:112, :])
                    for ic in range(2):
                        pp = ps_big.tile([128, 224], f32, tag="big")
                        for xc in range(4):
                            nc.tensor.matmul(
                                pp[0:112, :],
                                T_sb[0:112, xc, 112 * ic:112 * (ic + 1)],
                                axt[0:112, xc, 224 * t:224 * (t + 1)]
                                .rearrange("p (pj dx) -> p dx pj", dx=32),
                                start=(xc == 0), stop=(xc == 3))
                        pp_sb = work.tile([128, 224], bfl, tag="ppsb")
                        nc.vector.tensor_copy(pp_sb[0:112, :], pp[0:112, :])
                        eng = nc.gpsimd if (t % 2 == 0) else nc.sync
                        eng.dma_start(
                            S2[t, c, 112 * ic:112 * (ic + 1), :, :],
                            pp_sb[0:112, :].rearrange("i (dx pj) -> i dx pj", dx=32))

            # part im2col load [128, 24, 196] (n = t*49 + pi*7 + pj)
            rhs_p = res.tile([128, 24, 196], bfl, tag="imcol")
            for t in range(TOP_N):
                for c in range(3):
                    src_tc = S2[t, c].rearrange(
                        "(pi dyh dyl) dx pj -> dyh (dyl dx) pi pj",
                        dyh=8, dyl=4)
                    for dyh in range(8):
                        eng = nc.gpsimd if (dyh % 2 == 0) else nc.sync
                        eng.dma_start(
                            rhs_p[:, c * 8 + dyh, t * 49:(t + 1) * 49]
                            .rearrange("p (pi pj) -> p pi pj", pi=7),
                            src_tc[dyh])

            # ---------------- part backbone conv ----------------
            pf = res.tile([128, 16, 4], f32)
            po_sb = work.tile([128, 196], bfl, tag="posb")
            for m in range(16):
                pm = ps_big.tile([128, 196], f32, tag="big")
                for kc in range(24):
                    nc.tensor.matmul(pm[:], wbb_m[m // 2][:, kc, 128 * (m % 2):128 * (m % 2 + 1)],
                                     rhs_p[:, kc, :],
                                     start=(kc == 0), stop=(kc == 23))
                for t in range(TOP_N):
                    nc.scalar.activation(po_sb[:, 49 * t:49 * (t + 1)],
                                         pm[:, 49 * t:49 * (t + 1)], Act.Relu,
                                         accum_out=pf[:, m, t:t + 1])

            # ---------------- heads ----------------
            pfb = res.tile([128, 16, 4], bfl)
            nc.vector.tensor_copy(pfb[:], pf[:])


            plps = ps_sml.tile([4, 200], f32, tag="sml")
            for m4 in range(4):
                wc = whead.tile([128, 4, 200], bfl, tag="wcat")
                DMA(wc[:], p_wpart.rearrange("(mc p) n -> p mc n", p=128)
                    [:, 4 * m4:4 * (m4 + 1), :])
                for i in range(4):
                    m = 4 * m4 + i
                    nc.tensor.matmul(plps[:], pfb[:, m, :], wc[:, i, :],
                                     start=(m == 0), stop=(m == 15))
            plsb = work.tile([4, 200], f32, tag="hdsb")
            nc.vector.tensor_copy(plsb[:], plps[:])
            DMA(o_plog[:], plsb[:])

            catps = ps_sml.tile([1, 200], f32, tag="sml")
            for k4 in range(20):
                wc = whead.tile([128, 4, 200], bfl, tag="wcat")
                eng = nc.sync if k4 % 2 == 0 else nc.gpsimd
                eng.dma_start(wc[:], p_wcat.rearrange("(kc p) n -> p kc n", p=128)
                              [:, 4 * k4:4 * (k4 + 1), :])
                for i in range(4):
                    kc = 4 * k4 + i
                    if kc < 64:
                        t, m = divmod(kc, 16)
                        lhs = pfb[:, m, t:t + 1]
                    else:
                        m = kc - 64
                        lhs = featb[:, m:m + 1]
                    nc.tensor.matmul(catps[:], lhs, wc[:, i, :],
                                     start=(kc == 0), stop=(kc == 79))
            catsb = work.tile([1, 200], f32, tag="hdsb")
            nc.vector.tensor_copy(catsb[:], catps[:])
            DMA(o_cat[:], catsb[:])

    nc.compile()
    return nc


_NC = None
_RUN_KW = {}


def _get_nc():
    global _NC
    if _NC is None:
        _NC = build_kernel()
    return _NC


def kernel(**inputs):
    inputs = {k: np.asarray(v) for k, v in inputs.items()}
    nc = _get_nc()
    from concourse.bass_utils import run_bass_kernel_spmd

    W = _prep_weights(inputs)
    cst, crowc, ident = _host_consts()
    base = dict(W)
    base["cst"] = cst
    base["crow"] = crowc
    base["ident"] = ident
    in_maps = []
    for b in range(BATCH):
        m = dict(base)
        xb = inputs["x"][b]
        m["x"] = np.ascontiguousarray(xb).astype(bf16)
        m["xim"] = np.ascontiguousarray(
            xb.reshape(3, 14, 32, 14, 32).transpose(0, 2, 4, 1, 3)
            .reshape(3072, 196)).astype(bf16)
        in_maps.append(m)
    res = run_bass_kernel_spmd(nc, in_maps, list(range(BATCH)),
                               **_RUN_KW)
    r = res.results
    kernel.last_result = res
    raw = np.stack([r[b]["o_raw"][0] for b in range(BATCH)]).astype(np.float32)
    cat = np.stack([r[b]["o_cat"][0] for b in range(BATCH)]).astype(np.float32)
    plog = np.stack([r[b]["o_plog"] for b in range(BATCH)]).astype(np.float32)
    tidx = np.stack([r[b]["o_tidx"][0] for b in range(BATCH)])
    tprob = np.stack([r[b]["o_tprob"][0] for b in range(BATCH)]).astype(np.float32)
    tidx = np.rint(tidx).astype(np.int32)
    return raw, cat, plog, tidx, tprob


if __name__ == "__main__":
    d = np.load("/tmp/real_inputs.npz")
    out = kernel(**{k: d[k] for k in d.files})
    for o in out:
        print(o.shape, o.dtype)


# revision 24
# speedup vs baseline: 1.0368x; 1.0368x over previous
"""Trainium2 Bass kernel for nn_AttentionNet (NTS-Net style NMS detection).

Sharding: pure data parallel — batch 8 -> 1 image per NeuronCore (8 cores).
Everything for one image (backbone, proposal net, hard-NMS, crop-resize,
part backbone, heads) runs in a single Bass/Tile kernel on its core.

Key device-side tricks:
  * backbone conv (32x32 stride 32) == patch-embed matmul over im2col APs
  * hard-NMS: vector.max/max_index two-level argmax over a [128,13] score
    layout; anchor coords gathered by one-hot masked reduction; IoU row
    computed in-place with tensor_scalar ops (inter <= TH*union form)
  * crop-resize (align_corners bilinear) == two matmuls with triangular
    interpolation matrices relu(1 - |t - c|) built on device from the box
  * cropped parts bounce through DRAM to re-layout into im2col so the part
    backbone reuses the same resident W_bb and matmul loop
"""

import numpy as np
import ml_dtypes

bf16 = ml_dtypes.bfloat16

TOP_N = 4
PART = 224
IMG = 448
NUM_CLASS = 200
BATCH = 8
N_ANCH = 1614
NP_ANCH = 1664  # padded to 128*13
NEG = -1e30


# ---------------------------------------------------------------- host consts
def _make_edge_anchors():
    aspects = [0.667, 1.0, 1.5]
    settings = [
        (32, 14, 48.0, [2 ** (1.0 / 3), 2 ** (2.0 / 3)]),
        (64, 7, 96.0, [2 ** (1.0 / 3), 2 ** (2.0 / 3)]),
        (128, 4, 192.0, [1.0, 2 ** (1.0 / 3), 2 ** (2.0 / 3)]),
    ]
    boxes = []
    for stride, fm, size, scales in settings:
        for s in scales:
            for ar in aspects:
                h = size * s / np.sqrt(ar)
                w = size * s * np.sqrt(ar)
                c = (np.arange(fm) + 0.5) * stride
                cy, cx = np.meshgrid(c, c, indexing="ij")
                b = np.stack([cy - h / 2, cx - w / 2, cy + h / 2, cx + w / 2], -1)
                boxes.append(b.reshape(-1, 4))
    e = np.clip(np.concatenate(boxes, 0), 0, IMG)
    return e.astype(np.int64)  # UNPADDED coords [0,448]


EDGE_U = _make_edge_anchors()  # [1614,4] int


def _host_consts():
    # cst_f32 [128, 96]
    cst = np.zeros((128, 96), np.float32)
    p = np.arange(128)
    j = np.arange(13)
    cst[:, 0:13] = (p[:, None] * 13 + j[None, :]).astype(np.float32)  # iota2d
    A = EDGE_U.astype(np.float32)
    coords = np.zeros((NP_ANCH, 5), np.float32)
    coords[:N_ANCH, 0:4] = A
    coords[:N_ANCH, 4] = (A[:, 2] - A[:, 0]) * (A[:, 3] - A[:, 1])
    c2 = coords.reshape(128, 13, 5)
    for k in range(5):
        cst[:, 13 + 13 * k:13 + 13 * (k + 1)] = c2[:, :, k]
    cst[:, 78] = p.astype(np.float32)          # iota_p
    cst[:, 79] = 1.0                            # ones col
    for yc in range(4):
        cst[:, 80 + yc] = (p + 112 * yc).astype(np.float32)  # iota_pc
    # cst_row [1, 384]
    cr = np.zeros((1, 512), np.float32)
    cr[0, 0:224] = np.arange(224, dtype=np.float32)
    cr[0, 224:352] = np.arange(128, dtype=np.float32)
    cr[0, 384:512] = 1.0
    ident = np.eye(128, dtype=np.float32)
    return cst, cr, ident


def _prep_weights(inputs):
    """Host-side weight re-layouts (all bf16 except biases)."""
    W = {}
    W["wbb"] = np.ascontiguousarray(
        inputs["W_bb"].reshape(2048, 3 * 32 * 32).T).astype(bf16)  # [3072,2048]
    # W_d1 [128,2048,3,3] -> [16,3,3,128,128] (kc,ky,kx,p_in,m_out)
    wd1 = inputs["W_d1"].reshape(128, 16, 128, 3, 3)
    W["wd1"] = np.ascontiguousarray(wd1.transpose(1, 3, 4, 2, 0)).astype(bf16)
    for nm, key in (("wd2", "W_d2"), ("wd3", "W_d3")):
        w = inputs[key]  # [128,128,3,3]
        W[nm] = np.ascontiguousarray(w.transpose(2, 3, 1, 0)).astype(bf16)  # [3,3,128,128]
    W["wt1"] = np.ascontiguousarray(inputs["W_t1"][:, :, 0, 0].T).astype(bf16)  # [128,6]
    W["wt2"] = np.ascontiguousarray(inputs["W_t2"][:, :, 0, 0].T).astype(bf16)
    W["wt3"] = np.ascontiguousarray(inputs["W_t3"][:, :, 0, 0].T).astype(bf16)  # [128,9]
    # fold mean-pool scales into head weights
    W["wfc"] = np.ascontiguousarray((inputs["W_fc"] / 196.0).T).astype(bf16)  # [2048,200]
    W["wpart"] = np.ascontiguousarray((inputs["W_part"] / 49.0).T).astype(bf16)
    wcat = inputs["W_cat"].copy()  # [200, 10240]
    wcat[:, :8192] /= 49.0
    wcat[:, 8192:] /= 196.0
    W["wcat"] = np.ascontiguousarray(wcat.T).astype(bf16)  # [10240,200]
    bts = np.zeros((96, 1), np.float32)
    bts[0:6, 0] = inputs["b_t1"]
    bts[32:38, 0] = inputs["b_t2"]
    bts[64:73, 0] = inputs["b_t3"]
    W["bts"] = bts
    return W


# ---------------------------------------------------------------- bass kernel
def build_kernel():
    import concourse.bass as bass
    import concourse.mybir as mybir
    from concourse import bacc, tile

    f32 = mybir.dt.float32
    bfl = mybir.dt.bfloat16
    u32 = mybir.dt.uint32
    u8 = mybir.dt.uint8
    Alu = mybir.AluOpType
    Act = mybir.ActivationFunctionType
    AX = mybir.AxisListType

    nc = bacc.Bacc(None, target_bir_lowering=False)

    # ---- I/O ----
    x_in = nc.declare_dram_parameter("x", [3, 448, 448], bfl, isOutput=False)
    p_xim = nc.declare_dram_parameter("xim", [3072, 196], bfl, isOutput=False)
    p_wbb = nc.declare_dram_parameter("wbb", [3072, 2048], bfl, isOutput=False)
    p_wd1 = nc.declare_dram_parameter("wd1", [16, 3, 3, 128, 128], bfl, isOutput=False)
    p_wd2 = nc.declare_dram_parameter("wd2", [3, 3, 128, 128], bfl, isOutput=False)
    p_wd3 = nc.declare_dram_parameter("wd3", [3, 3, 128, 128], bfl, isOutput=False)
    p_wt1 = nc.declare_dram_parameter("wt1", [128, 6], bfl, isOutput=False)
    p_wt2 = nc.declare_dram_parameter("wt2", [128, 6], bfl, isOutput=False)
    p_wt3 = nc.declare_dram_parameter("wt3", [128, 9], bfl, isOutput=False)
    p_wfc = nc.declare_dram_parameter("wfc", [2048, 200], bfl, isOutput=False)
    p_wpart = nc.declare_dram_parameter("wpart", [2048, 200], bfl, isOutput=False)
    p_wcat = nc.declare_dram_parameter("wcat", [10240, 200], bfl, isOutput=False)
    p_bts = nc.declare_dram_parameter("bts", [96, 1], f32, isOutput=False)
    p_cst = nc.declare_dram_parameter("cst", [128, 96], f32, isOutput=False)
    p_crow = nc.declare_dram_parameter("crow", [1, 512], f32, isOutput=False)
    p_ident = nc.declare_dram_parameter("ident", [128, 128], f32, isOutput=False)

    o_raw = nc.declare_dram_parameter("o_raw", [1, 200], f32, isOutput=True)
    o_cat = nc.declare_dram_parameter("o_cat", [1, 200], f32, isOutput=True)
    o_plog = nc.declare_dram_parameter("o_plog", [4, 200], f32, isOutput=True)
    o_tidx = nc.declare_dram_parameter("o_tidx", [1, 4], f32, isOutput=True)
    o_tprob = nc.declare_dram_parameter("o_tprob", [1, 4], f32, isOutput=True)

    with tile.TileContext(nc) as tc:
        with (
            tc.tile_pool(name="res", bufs=1) as res,           # resident
            tc.tile_pool(name="wstr", bufs=2) as wstr,
            tc.tile_pool(name="whead", bufs=8) as whead,         # streamed weights
            tc.tile_pool(name="work", bufs=3) as work,         # working tiles
            tc.tile_pool(name="nms", bufs=1) as nmsp,          # nms state
            tc.tile_pool(name="ps_big", bufs=4, space="PSUM") as ps_big,
            tc.tile_pool(name="ps_sml", bufs=2, space="PSUM") as ps_sml,
            tc.tile_pool(name="dram", bufs=1, space="DRAM") as dpool,
        ):
            DMA = nc.sync.dma_start

            # ---------------- resident loads ----------------
            cst = res.tile([128, 96], f32)
            DMA(cst[:], p_cst[:])
            crow = res.tile([1, 512], f32)
            DMA(crow[:], p_crow[:])
            ident = res.tile([128, 128], f32)
            DMA(ident[:], p_ident[:])
            bts = res.tile([96, 1], f32)
            DMA(bts[:], p_bts[:])

            iota2d = cst[:, 0:13]
            y0a = cst[:, 13:26]
            x0a = cst[:, 26:39]
            y1a = cst[:, 39:52]
            x1a = cst[:, 52:65]
            area_a = cst[:, 65:78]
            ones_col = cst[:, 79:80]
            iota224 = crow[:, 0:224]
            iota128r = crow[:, 224:352]
            ones_row = crow[:, 384:512]

            im2col = res.tile([128, 24, 196], bfl, tag="imcol")
            DMA(im2col[:], p_xim.rearrange("(kc p) n -> p kc n", p=128))
            wbb_src = p_wbb.rearrange("(kc p) m -> p kc m", p=128)
            wbb_m = []
            for m in range(8):
                wt = res.tile([128, 24, 256], bfl, tag=f"wbb{m}")
                eng = nc.sync if m % 2 == 0 else nc.gpsimd
                eng.dma_start(wt[:], wbb_src[:, :, 256 * m:256 * (m + 1)])
                wbb_m.append(wt)
            wt1 = res.tile([128, 6], bfl)
            DMA(wt1[:], p_wt1[:])
            wt2 = res.tile([128, 6], bfl)
            DMA(wt2[:], p_wt2[:])
            wt3 = res.tile([128, 9], bfl)
            DMA(wt3[:], p_wt3[:])
            wd2 = res.tile([128, 9, 128], bfl)
            DMA(wd2[:], p_wd2.rearrange("ky kx p m -> p (ky kx) m"))
            wd3 = res.tile([128, 9, 128], bfl)
            DMA(wd3[:], p_wd3.rearrange("ky kx p m -> p (ky kx) m"))

            # image, [y-chunks] layout for crop matmuls (lhsT)
            img_yx = res.tile([128, 3, 4, 448], bfl)
            nc.gpsimd.dma_start(img_yx[0:112, :, :, :],
                x_in.rearrange("c (yc p) x -> p c yc x", p=112))

            # ---------------- main backbone conv ----------------
            f_sb = res.tile([128, 16, 196], bfl)
            feat = res.tile([128, 16], f32)
            for m in range(16):
                pm = ps_big.tile([128, 196], f32, tag="big")
                for kc in range(24):
                    nc.tensor.matmul(pm[:], wbb_m[m // 2][:, kc, 128 * (m % 2):128 * (m % 2 + 1)],
                                     im2col[:, kc, :],
                                     start=(kc == 0), stop=(kc == 23))
                nc.scalar.activation(f_sb[:, m, :], pm[:], Act.Relu,
                                     accum_out=feat[:, m:m + 1])

            # ---------------- proposal net ----------------
            fpad = res.tile([128, 16, 16, 16], bfl)
            nc.vector.memset(fpad[:], 0.0)
            nc.vector.tensor_copy(
                fpad[:, :, 1:15, 1:15],
                f_sb.rearrange("p m (y x) -> p m y x", y=14))
            d1ps = ps_big.tile([128, 196], f32, tag="big")
            for kc in range(16):
                wchunk = wstr.tile([128, 9, 128], bfl, tag="wd1")
                eng = nc.sync if kc % 2 == 0 else nc.gpsimd
                eng.dma_start(wchunk[:], p_wd1[kc].rearrange("ky kx p m -> p (ky kx) m"))
                for t9 in range(9):
                    ky, kx = divmod(t9, 3)
                    nc.tensor.matmul(
                        d1ps[:],
                        wchunk[:, t9, :],
                        fpad[:, kc, ky:ky + 14, kx:kx + 14],
                        start=(kc == 0 and t9 == 0), stop=(kc == 15 and t9 == 8))
            d1pad = res.tile([128, 16, 16], bfl)
            nc.vector.memset(d1pad[:], 0.0)
            nc.scalar.activation(
                d1pad[:, 1:15, 1:15],
                d1ps.rearrange("p (y x) -> p y x", y=14)[:],
                Act.Relu)

            t1ps = ps_sml.tile([6, 196], f32, tag="sml")
            nc.tensor.matmul(t1ps[:], wt1[:],
                             d1pad[:, 1:15, 1:15], start=True, stop=True)
            t1sb = work.tile([6, 196], f32, tag="tsb")
            nc.vector.tensor_scalar_add(t1sb[:], t1ps[:], bts[0:6, :])

            d2ps = ps_sml.tile([128, 49], f32, tag="sml")
            for t9 in range(9):
                ky, kx = divmod(t9, 3)
                nc.tensor.matmul(
                    d2ps[:], wd2[:, t9, :],
                    d1pad[:, ky:ky + 14:2, kx:kx + 14:2],
                    start=(t9 == 0), stop=(t9 == 8))
            d2pad = res.tile([128, 9, 9], bfl)
            nc.vector.memset(d2pad[:], 0.0)
            nc.scalar.activation(d2pad[:, 1:8, 1:8],
                                 d2ps.rearrange("p (y x) -> p y x", y=7)[:],
                                 Act.Relu)
            t2ps = ps_sml.tile([6, 49], f32, tag="sml")
            nc.tensor.matmul(t2ps[:], wt2[:], d2pad[:, 1:8, 1:8],
                             start=True, stop=True)
            t2sb = work.tile([6, 49], f32, tag="tsb")
            nc.vector.tensor_scalar_add(t2sb[:], t2ps[:], bts[32:38, :])

            d3ps = ps_sml.tile([128, 16], f32, tag="sml")
            for t9 in range(9):
                ky, kx = divmod(t9, 3)
                nc.tensor.matmul(
                    d3ps[:], wd3[:, t9, :],
                    d2pad[:, ky:ky + 7:2, kx:kx + 7:2],
                    start=(t9 == 0), stop=(t9 == 8))
            d3sb = work.tile([128, 16], bfl, tag="d3sb")
            nc.scalar.activation(d3sb[:], d3ps[:], Act.Relu)
            t3ps = ps_sml.tile([9, 16], f32, tag="sml")
            nc.tensor.matmul(t3ps[:], wt3[:], d3sb[:], start=True, stop=True)
            t3sb = work.tile([9, 16], f32, tag="tsb")
            nc.vector.tensor_scalar_add(t3sb[:], t3ps[:], bts[64:73, :])

            # assemble scores through DRAM bounce -> [128,13]
            sc_d = dpool.tile([1664], f32)
            DMA(sc_d[0:1176].rearrange("(a n) -> a n", a=6), t1sb[:])
            DMA(sc_d[1176:1470].rearrange("(a n) -> a n", a=6), t2sb[:])
            DMA(sc_d[1470:1614].rearrange("(a n) -> a n", a=9), t3sb[:])
            padt = work.tile([1, 50], f32, tag="padt")
            nc.vector.memset(padt[:], NEG)
            DMA(sc_d[1614:1664].rearrange("(a n) -> a n", a=1), padt[:])
            scores = nmsp.tile([128, 13], f32)
            DMA(scores[:], sc_d.rearrange("(p j) -> p j", p=128))

            # ---------------- NMS (4 rounds) ----------------
            neginf2d = nmsp.tile([128, 13], f32)
            nc.vector.memset(neginf2d[:], NEG)
            topidx = nmsp.tile([1, 4], f32)
            topprob = nmsp.tile([1, 4], f32)
            boxes = nmsp.tile([1, 16], f32)
            scratch = nmsp.tile([128, 13], f32)
            iy_t = nmsp.tile([128, 13], f32)
            ix_t = nmsp.tile([128, 13], f32)
            un_t = nmsp.tile([128, 13], f32)
            supp = nmsp.tile([128, 13], u8)
            csum = nmsp.tile([128, 5], f32)

            ty_all = nmsp.tile([1, 896], f32)
            tx_all = nmsp.tile([1, 896], f32)
            sm = nmsp.tile([1, 8], f32)
            ayt = res.tile([128, 4, 896], bfl)
            axt = res.tile([128, 4, 896], bfl)
            wtmp32 = nmsp.tile([128, 896], f32)
            wtmp = nmsp.tile([128, 896], bfl)
            S2 = dpool.tile([4, 3, 224, 32, 7], bfl)  # [t, c, i, dx, pj]
            for t in range(TOP_N):
                pm8 = nmsp.tile([128, 8], f32, tag="pm8")
                nc.vector.max(pm8[:], scores[:])
                fi8 = nmsp.tile([128, 8], u32, tag="fi8")
                nc.vector.max_index(fi8[:], pm8[:], scores[:])
                fjf = nmsp.tile([128, 1], f32, tag="fjf")
                nc.vector.tensor_copy(fjf[:], fi8[:, 0:1])
                trpsA = ps_sml.tile([1, 128], f32, tag="sml")
                nc.tensor.transpose(trpsA[:], pm8[:, 0:1], ident[:])
                trowA = nmsp.tile([1, 128], f32, tag="trowA")
                nc.vector.tensor_copy(trowA[:], trpsA[:])
                trpsB = ps_sml.tile([1, 128], f32, tag="sml")
                nc.tensor.transpose(trpsB[:], fjf[:], ident[:])
                trowB = nmsp.tile([1, 128], f32, tag="trowB")
                nc.vector.tensor_copy(trowB[:], trpsB[:])
                g8 = nmsp.tile([1, 8], f32, tag="g8")
                nc.vector.max(g8[:], trowA[:])
                nc.vector.tensor_copy(topprob[:, t:t + 1], g8[:, 0:1])
                gi8 = nmsp.tile([1, 8], u32, tag="gi8")
                nc.vector.max_index(gi8[:], g8[:], trowA[:])
                pstar = nmsp.tile([1, 2], f32, tag="pstar")
                nc.vector.tensor_copy(pstar[:, 0:1], gi8[:, 0:1])
                # jstar = jrow[p*]
                mrow = nmsp.tile([1, 128], f32, tag="mrow")
                nc.vector.scalar_tensor_tensor(
                    mrow[:], iota128r, pstar[:, 0:1], trowB[:],
                    Alu.is_equal, Alu.mult, accum_out=pstar[:, 1:2])
                # flat = p*13 + j
                nc.vector.scalar_tensor_tensor(
                    topidx[:, t:t + 1], pstar[:, 0:1], 13.0, pstar[:, 1:2],
                    Alu.mult, Alu.add)
                # broadcast flat to [128,1]
                fb_ps = ps_sml.tile([128, 1], f32, tag="sml")
                nc.tensor.matmul(fb_ps[:], ones_row, topidx[:, t:t + 1],
                                 start=True, stop=True)
                flatb = nmsp.tile([128, 1], f32, tag="flatb")
                nc.vector.tensor_copy(flatb[:], fb_ps[:])
                # gather 5 coords of picked anchor
                for k, cv in enumerate((y0a, x0a, y1a, x1a, area_a)):
                    nc.vector.scalar_tensor_tensor(
                        scratch[:], iota2d, flatb[:], cv,
                        Alu.is_equal, Alu.mult, accum_out=csum[:, k:k + 1])
                s5ps = ps_sml.tile([1, 5], f32, tag="sml")
                nc.tensor.matmul(s5ps[:], ones_col, csum[:], start=True, stop=True)
                s5 = nmsp.tile([1, 5], f32, tag="s5")
                nc.vector.tensor_copy(s5[:], s5ps[:])
                nc.vector.tensor_copy(boxes[:, 4 * t:4 * t + 4], s5[:, 0:4])
                # ---- part t crop (overlaps later NMS rounds) ----
                y0b = boxes[:, 4 * t:4 * t + 1]
                x0b = boxes[:, 4 * t + 1:4 * t + 2]
                y1b = boxes[:, 4 * t + 2:4 * t + 3]
                x1b = boxes[:, 4 * t + 3:4 * t + 4]
                nc.vector.scalar_tensor_tensor(
                    sm[:, 2 * t:2 * t + 1], y1b, -1.0, y0b, Alu.add, Alu.subtract)
                nc.vector.tensor_scalar_mul(
                    sm[:, 2 * t:2 * t + 1], sm[:, 2 * t:2 * t + 1], 1.0 / 223.0)
                nc.vector.tensor_scalar(
                    ty_all[:, 224 * t:224 * (t + 1)], iota224,
                    sm[:, 2 * t:2 * t + 1], y0b, Alu.mult, Alu.add)
                nc.vector.scalar_tensor_tensor(
                    sm[:, 2 * t + 1:2 * t + 2], x1b, -1.0, x0b, Alu.add, Alu.subtract)
                nc.vector.tensor_scalar_mul(
                    sm[:, 2 * t + 1:2 * t + 2], sm[:, 2 * t + 1:2 * t + 2], 1.0 / 223.0)
                nc.vector.tensor_scalar(
                    tx_all[:, 224 * t:224 * (t + 1)], iota224,
                    sm[:, 2 * t + 1:2 * t + 2], x0b, Alu.mult, Alu.add)
                bpsy = ps_sml.tile([128, 224], f32, tag="sml")
                nc.tensor.matmul(bpsy[:], ones_row,
                                 ty_all[:, 224 * t:224 * (t + 1)], start=True, stop=True)
                tyb_t = nmsp.tile([128, 224], f32, tag="tybt")
                nc.vector.tensor_copy(tyb_t[:], bpsy[:])
                bpsx = ps_sml.tile([128, 224], f32, tag="sml")
                nc.tensor.matmul(bpsx[:], ones_row,
                                 tx_all[:, 224 * t:224 * (t + 1)], start=True, stop=True)
                txb_t = nmsp.tile([128, 224], f32, tag="txbt")
                nc.vector.tensor_copy(txb_t[:], bpsx[:])
                for yc in range(4):
                    nc.vector.tensor_scalar_sub(wtmp32[:, 0:224], tyb_t[:],
                                                cst[:, 80 + yc:81 + yc])
                    nc.scalar.activation(wtmp[:, 0:224], wtmp32[:, 0:224], Act.Abs)
                    nc.scalar.activation(ayt[:, yc, 224 * t:224 * (t + 1)],
                                         wtmp[:, 0:224], Act.Relu,
                                         bias=ones_col, scale=-1.0)
                    nc.vector.tensor_scalar_sub(wtmp32[:, 0:224], txb_t[:],
                                                cst[:, 80 + yc:81 + yc])
                    nc.scalar.activation(wtmp[:, 0:224], wtmp32[:, 0:224], Act.Abs)
                    nc.scalar.activation(axt[:, yc, 224 * t:224 * (t + 1)],
                                         wtmp[:, 0:224], Act.Relu,
                                         bias=ones_col, scale=-1.0)
                for c in range(3):
                    T_sb = work.tile([128, 4, 224], bfl, tag="Tsb")
                    for xc in range(4):
                        tp = ps_big.tile([128, 224], f32, tag="big")
                        for yc in range(4):
                            nc.tensor.matmul(
                                tp[0:112, :],
                                img_yx[0:112, c, yc, xc * 112:(xc + 1) * 112],
                                ayt[0:112, yc, 224 * t:224 * (t + 1)],
                                start=(yc == 0), stop=(yc == 3))
                        nc.vector.tensor_copy(T_sb[0:112, xc, :], tp[0:112, :])
                    for ic in range(2):
                        pp = ps_big.tile([128, 224], f32, tag="big")
                        for xc in range(4):
                            nc.tensor.matmul(
                                pp[0:112, :],
                                T_sb[0:112, xc, 112 * ic:112 * (ic + 1)],
                                axt[0:112, xc, 224 * t:224 * (t + 1)]
                                .rearrange("p (pj dx) -> p dx pj", dx=32),
                                start=(xc == 0), stop=(xc == 3))
                        pp_sb = work.tile([128, 224], bfl, tag="ppsb")
                        nc.vector.tensor_copy(pp_sb[0:112, :], pp[0:112, :])
                        eng = nc.gpsimd if (t % 2 == 0) else nc.sync
                        eng.dma_start(
                            S2[t, c, 112 * ic:112 * (ic + 1), :, :],
                            pp_sb[0:112, :].rearrange("i (dx pj) -> i dx pj", dx=32))

                s5b_ps = ps_sml.tile([128, 5], f32, tag="sml")
                nc.tensor.matmul(s5b_ps[:], ones_row, s5[:], start=True, stop=True)
                s5b = nmsp.tile([128, 5], f32, tag="s5b")
                nc.vector.tensor_copy(s5b[:], s5b_ps[:])
                # IoU row and suppression
                nc.vector.tensor_scalar_max(scratch[:], y0a, s5b[:, 0:1])
                nc.vector.scalar_tensor_tensor(
                    iy_t[:], y1a, s5b[:, 2:3], scratch[:], Alu.min, Alu.subtract)
                nc.vector.tensor_scalar_max(iy_t[:], iy_t[:], 0.0)
                nc.vector.tensor_scalar_max(scratch[:], x0a, s5b[:, 1:2])
                nc.vector.scalar_tensor_tensor(
                    ix_t[:], x1a, s5b[:, 3:4], scratch[:], Alu.min, Alu.subtract)
                nc.vector.tensor_scalar_max(ix_t[:], ix_t[:], 0.0)
                nc.vector.tensor_tensor(ix_t[:], iy_t[:], ix_t[:], Alu.mult)  # inter
                nc.vector.scalar_tensor_tensor(
                    un_t[:], area_a, s5b[:, 4:5], ix_t[:], Alu.add, Alu.subtract)
                # suppress where 0.25*union < inter
                nc.vector.scalar_tensor_tensor(
                    supp[:], un_t[:], 0.25, ix_t[:], Alu.mult, Alu.is_lt)
                nc.vector.copy_predicated(scores[:], supp[:], neginf2d[:])

            DMA(o_tidx[:], topidx[:])
            DMA(o_tprob[:], topprob[:])

            # part im2col load [128, 24, 196] (n = t*49 + pi*7 + pj)
            rhs_p = res.tile([128, 24, 196], bfl, tag="imcol")
            for t in range(TOP_N):
                for c in range(3):
                    src_tc = S2[t, c].rearrange(
                        "(pi dyh dyl) dx pj -> dyh (dyl dx) pi pj",
                        dyh=8, dyl=4)
                    for dyh in range(8):
                        eng = nc.gpsimd if (dyh % 2 == 0) else nc.sync
                        eng.dma_start(
                            rhs_p[:, c * 8 + dyh, t * 49:(t + 1) * 49]
                            .rearrange("p (pi pj) -> p pi pj", pi=7),
                            src_tc[dyh])

            # ---------------- part backbone conv ----------------
            pf = res.tile([128, 16, 4], f32)
            po_sb = work.tile([128, 196], bfl, tag="posb")
            for m in range(16):
                pm = ps_big.tile([128, 196], f32, tag="big")
                for kc in range(24):
                    nc.tensor.matmul(pm[:], wbb_m[m // 2][:, kc, 128 * (m % 2):128 * (m % 2 + 1)],
                                     rhs_p[:, kc, :],
                                     start=(kc == 0), stop=(kc == 23))
                for t in range(TOP_N):
                    nc.scalar.activation(po_sb[:, 49 * t:49 * (t + 1)],
                                         pm[:, 49 * t:49 * (t + 1)], Act.Relu,
                                         accum_out=pf[:, m, t:t + 1])

            # ---------------- heads ----------------
            featb = res.tile([128, 16], bfl)
            nc.vector.tensor_copy(featb[:], feat[:])
            pfb = res.tile([128, 16, 4], bfl)
            nc.vector.tensor_copy(pfb[:], pf[:])

            rawps = ps_sml.tile([1, 200], f32, tag="sml")
            for m4 in range(4):
                wc = whead.tile([128, 4, 200], bfl, tag="wcat")
                DMA(wc[:], p_wfc.rearrange("(mc p) n -> p mc n", p=128)
                    [:, 4 * m4:4 * (m4 + 1), :])
                for i in range(4):
                    m = 4 * m4 + i
                    nc.tensor.matmul(rawps[:], featb[:, m:m + 1], wc[:, i, :],
                                     start=(m == 0), stop=(m == 15))
            rawsb = work.tile([1, 200], f32, tag="hdsb")
            nc.vector.tensor_copy(rawsb[:], rawps[:])
            DMA(o_raw[:], rawsb[:])

            plps = ps_sml.tile([4, 200], f32, tag="sml")
            for m4 in range(4):
                wc = whead.tile([128, 4, 200], bfl, tag="wcat")
                DMA(wc[:], p_wpart.rearrange("(mc p) n -> p mc n", p=128)
                    [:, 4 * m4:4 * (m4 + 1), :])
                for i in range(4):
                    m = 4 * m4 + i
                    nc.tensor.matmul(plps[:], pfb[:, m, :], wc[:, i, :],
                                     start=(m == 0), stop=(m == 15))
            plsb = work.tile([4, 200], f32, tag="hdsb")
            nc.vector.tensor_copy(plsb[:], plps[:])
            DMA(o_plog[:], plsb[:])

            catps = ps_sml.tile([1, 200], f32, tag="sml")
            for k4 in range(20):
                wc = whead.tile([128, 4, 200], bfl, tag="wcat")
                eng = nc.sync if k4 % 2 == 0 else nc.gpsimd
                eng.dma_start(wc[:], p_wcat.rearrange("(kc p) n -> p kc n", p=128)
                              [:, 4 * k4:4 * (k4 + 1), :])
                for i in range(4):
                    kc = 4 * k4 + i
                    if kc < 64:
                        t, m = divmod(kc, 16)
                        lhs = pfb[:, m, t:t + 1]
                    else:
                        m = kc - 64
                        lhs = featb[:, m:m + 1]
                    nc.tensor.matmul(catps[:], lhs, wc[:, i, :],
                                     start=(kc == 0), stop=(kc == 79))
            catsb = work.tile([1, 200], f32, tag="hdsb")
            nc.vector.tensor_copy(catsb[:], catps[:])
            DMA(o_cat[:], catsb[:])

    nc.compile()
    return nc


_NC = None
_RUN_KW = {}


def _get_nc():
    global _NC
    if _NC is None:
        _NC = build_kernel()
    return _NC


def kernel(**inputs):
    inputs = {k: np.asarray(v) for k, v in inputs.items()}
    nc = _get_nc()
    from concourse.bass_utils import run_bass_kernel_spmd

    W = _prep_weights(inputs)
    cst, crowc, ident = _host_consts()
    base = dict(W)
    base["cst"] = cst
    base["crow"] = crowc
    base["ident"] = ident
    in_maps = []
    for b in range(BATCH):
        m = dict(base)
        xb = inputs["x"][b]
        m["x"] = np.ascontiguousarray(xb).astype(bf16)
        m["xim"] = np.ascontiguousarray(
            xb.reshape(3, 14, 32, 14, 32).transpose(0, 2, 4, 1, 3)
            .reshape(3072, 196)).astype(bf16)
        in_maps.append(m)
    res = run_bass_kernel_spmd(nc, in_maps, list(range(BATCH)),
                               **_RUN_KW)
    r = res.results
    kernel.last_result = res
    raw = np.stack([r[b]["o_raw"][0] for b in range(BATCH)]).astype(np.float32)
    cat = np.stack([r[b]["o_cat"][0] for b in range(BATCH)]).astype(np.float32)
    plog = np.stack([r[b]["o_plog"] for b in range(BATCH)]).astype(np.float32)
    tidx = np.stack([r[b]["o_tidx"][0] for b in range(BATCH)])
    tprob = np.stack([r[b]["o_tprob"][0] for b in range(BATCH)]).astype(np.float32)
    tidx = np.rint(tidx).astype(np.int32)
    return raw, cat, plog, tidx, tprob


if __name__ == "__main__":
    d = np.load("/tmp/real_inputs.npz")
    out = kernel(**{k: d[k] for k in d.files})
    for o in out:
        print(o.shape, o.dtype)


# revision 25
# speedup vs baseline: 1.0970x; 1.0580x over previous
"""Trainium2 Bass kernel for nn_AttentionNet (NTS-Net style NMS detection).

Sharding: pure data parallel — batch 8 -> 1 image per NeuronCore (8 cores).
Everything for one image (backbone, proposal net, hard-NMS, crop-resize,
part backbone, heads) runs in a single Bass/Tile kernel on its core.

Key device-side tricks:
  * backbone conv (32x32 stride 32) == patch-embed matmul over im2col APs
  * hard-NMS: vector.max/max_index two-level argmax over a [128,13] score
    layout; anchor coords gathered by one-hot masked reduction; IoU row
    computed in-place with tensor_scalar ops (inter <= TH*union form)
  * crop-resize (align_corners bilinear) == two matmuls with triangular
    interpolation matrices relu(1 - |t - c|) built on device from the box
  * cropped parts bounce through DRAM to re-layout into im2col so the part
    backbone reuses the same resident W_bb and matmul loop
"""

import numpy as np
import ml_dtypes

bf16 = ml_dtypes.bfloat16

TOP_N = 4
PART = 224
IMG = 448
NUM_CLASS = 200
BATCH = 8
N_ANCH = 1614
NP_ANCH = 1664  # padded to 128*13
NEG = -1e30


# ---------------------------------------------------------------- host consts
def _make_edge_anchors():
    aspects = [0.667, 1.0, 1.5]
    settings = [
        (32, 14, 48.0, [2 ** (1.0 / 3), 2 ** (2.0 / 3)]),
        (64, 7, 96.0, [2 ** (1.0 / 3), 2 ** (2.0 / 3)]),
        (128, 4, 192.0, [1.0, 2 ** (1.0 / 3), 2 ** (2.0 / 3)]),
    ]
    boxes = []
    for stride, fm, size, scales in settings:
        for s in scales:
            for ar in aspects:
                h = size * s / np.sqrt(ar)
                w = size * s * np.sqrt(ar)
                c = (np.arange(fm) + 0.5) * stride
                cy, cx = np.meshgrid(c, c, indexing="ij")
                b = np.stack([cy - h / 2, cx - w / 2, cy + h / 2, cx + w / 2], -1)
                boxes.append(b.reshape(-1, 4))
    e = np.clip(np.concatenate(boxes, 0), 0, IMG)
    return e.astype(np.int64)  # UNPADDED coords [0,448]


EDGE_U = _make_edge_anchors()  # [1614,4] int


def _host_consts():
    # cst_f32 [128, 96]
    cst = np.zeros((128, 96), np.float32)
    p = np.arange(128)
    j = np.arange(13)
    cst[:, 0:13] = (p[:, None] * 13 + j[None, :]).astype(np.float32)  # iota2d
    A = EDGE_U.astype(np.float32)
    coords = np.zeros((NP_ANCH, 5), np.float32)
    coords[:N_ANCH, 0:4] = A
    coords[:N_ANCH, 4] = (A[:, 2] - A[:, 0]) * (A[:, 3] - A[:, 1])
    c2 = coords.reshape(128, 13, 5)
    for k in range(5):
        cst[:, 13 + 13 * k:13 + 13 * (k + 1)] = c2[:, :, k]
    cst[:, 78] = p.astype(np.float32)          # iota_p
    cst[:, 79] = 1.0                            # ones col
    for yc in range(4):
        cst[:, 80 + yc] = (p + 112 * yc).astype(np.float32)  # iota_pc
    # cst_row [1, 384]
    cr = np.zeros((1, 512), np.float32)
    cr[0, 0:224] = np.arange(224, dtype=np.float32)
    cr[0, 224:352] = np.arange(128, dtype=np.float32)
    cr[0, 384:512] = 1.0
    ident = np.eye(128, dtype=np.float32)
    return cst, cr, ident


def _prep_weights(inputs):
    """Host-side weight re-layouts (all bf16 except biases)."""
    W = {}
    W["wbb"] = np.ascontiguousarray(
        inputs["W_bb"].reshape(2048, 3 * 32 * 32).T).astype(bf16)  # [3072,2048]
    # W_d1 [128,2048,3,3] -> [16,3,3,128,128] (kc,ky,kx,p_in,m_out)
    wd1 = inputs["W_d1"].reshape(128, 16, 128, 3, 3)
    W["wd1"] = np.ascontiguousarray(wd1.transpose(1, 3, 4, 2, 0)).astype(bf16)
    for nm, key in (("wd2", "W_d2"), ("wd3", "W_d3")):
        w = inputs[key]  # [128,128,3,3]
        W[nm] = np.ascontiguousarray(w.transpose(2, 3, 1, 0)).astype(bf16)  # [3,3,128,128]
    W["wt1"] = np.ascontiguousarray(inputs["W_t1"][:, :, 0, 0].T).astype(bf16)  # [128,6]
    W["wt2"] = np.ascontiguousarray(inputs["W_t2"][:, :, 0, 0].T).astype(bf16)
    W["wt3"] = np.ascontiguousarray(inputs["W_t3"][:, :, 0, 0].T).astype(bf16)  # [128,9]
    # fold mean-pool scales into head weights
    W["wfc"] = np.ascontiguousarray((inputs["W_fc"] / 196.0).T).astype(bf16)  # [2048,200]
    W["wpart"] = np.ascontiguousarray((inputs["W_part"] / 49.0).T).astype(bf16)
    wcat = inputs["W_cat"].copy()  # [200, 10240]
    wcat[:, :8192] /= 49.0
    wcat[:, 8192:] /= 196.0
    W["wcat"] = np.ascontiguousarray(wcat.T).astype(bf16)  # [10240,200]
    bts = np.zeros((96, 1), np.float32)
    bts[0:6, 0] = inputs["b_t1"]
    bts[32:38, 0] = inputs["b_t2"]
    bts[64:73, 0] = inputs["b_t3"]
    W["bts"] = bts
    return W


# ---------------------------------------------------------------- bass kernel
def build_kernel():
    import concourse.bass as bass
    import concourse.mybir as mybir
    from concourse import bacc, tile

    f32 = mybir.dt.float32
    bfl = mybir.dt.bfloat16
    u32 = mybir.dt.uint32
    u8 = mybir.dt.uint8
    Alu = mybir.AluOpType
    Act = mybir.ActivationFunctionType
    AX = mybir.AxisListType

    nc = bacc.Bacc(None, target_bir_lowering=False)

    # ---- I/O ----
    x_in = nc.declare_dram_parameter("x", [3, 448, 448], bfl, isOutput=False)
    p_xim = nc.declare_dram_parameter("xim", [3072, 196], bfl, isOutput=False)
    p_wbb = nc.declare_dram_parameter("wbb", [3072, 2048], bfl, isOutput=False)
    p_wd1 = nc.declare_dram_parameter("wd1", [16, 3, 3, 128, 128], bfl, isOutput=False)
    p_wd2 = nc.declare_dram_parameter("wd2", [3, 3, 128, 128], bfl, isOutput=False)
    p_wd3 = nc.declare_dram_parameter("wd3", [3, 3, 128, 128], bfl, isOutput=False)
    p_wt1 = nc.declare_dram_parameter("wt1", [128, 6], bfl, isOutput=False)
    p_wt2 = nc.declare_dram_parameter("wt2", [128, 6], bfl, isOutput=False)
    p_wt3 = nc.declare_dram_parameter("wt3", [128, 9], bfl, isOutput=False)
    p_wfc = nc.declare_dram_parameter("wfc", [2048, 200], bfl, isOutput=False)
    p_wpart = nc.declare_dram_parameter("wpart", [2048, 200], bfl, isOutput=False)
    p_wcat = nc.declare_dram_parameter("wcat", [10240, 200], bfl, isOutput=False)
    p_bts = nc.declare_dram_parameter("bts", [96, 1], f32, isOutput=False)
    p_cst = nc.declare_dram_parameter("cst", [128, 96], f32, isOutput=False)
    p_crow = nc.declare_dram_parameter("crow", [1, 512], f32, isOutput=False)
    p_ident = nc.declare_dram_parameter("ident", [128, 128], f32, isOutput=False)

    o_raw = nc.declare_dram_parameter("o_raw", [1, 200], f32, isOutput=True)
    o_cat = nc.declare_dram_parameter("o_cat", [1, 200], f32, isOutput=True)
    o_plog = nc.declare_dram_parameter("o_plog", [4, 200], f32, isOutput=True)
    o_tidx = nc.declare_dram_parameter("o_tidx", [1, 4], f32, isOutput=True)
    o_tprob = nc.declare_dram_parameter("o_tprob", [1, 4], f32, isOutput=True)

    with tile.TileContext(nc) as tc:
        with (
            tc.tile_pool(name="res", bufs=1) as res,           # resident
            tc.tile_pool(name="wstr", bufs=3) as wstr,
            tc.tile_pool(name="whead", bufs=8) as whead,         # streamed weights
            tc.tile_pool(name="work", bufs=4) as work,         # working tiles
            tc.tile_pool(name="nms", bufs=1) as nmsp,          # nms state
            tc.tile_pool(name="ps_big", bufs=6, space="PSUM") as ps_big,
            tc.tile_pool(name="ps_sml", bufs=2, space="PSUM") as ps_sml,
            tc.tile_pool(name="dram", bufs=1, space="DRAM") as dpool,
        ):
            DMA = nc.sync.dma_start

            # ---------------- resident loads ----------------
            cst = res.tile([128, 96], f32)
            DMA(cst[:], p_cst[:])
            crow = res.tile([1, 512], f32)
            DMA(crow[:], p_crow[:])
            ident = res.tile([128, 128], f32)
            DMA(ident[:], p_ident[:])
            bts = res.tile([96, 1], f32)
            DMA(bts[:], p_bts[:])

            iota2d = cst[:, 0:13]
            y0a = cst[:, 13:26]
            x0a = cst[:, 26:39]
            y1a = cst[:, 39:52]
            x1a = cst[:, 52:65]
            area_a = cst[:, 65:78]
            ones_col = cst[:, 79:80]
            iota224 = crow[:, 0:224]
            iota128r = crow[:, 224:352]
            ones_row = crow[:, 384:512]

            im2col = res.tile([128, 24, 196], bfl, tag="imcol")
            DMA(im2col[:], p_xim.rearrange("(kc p) n -> p kc n", p=128))
            wbb_src = p_wbb.rearrange("(kc p) m -> p kc m", p=128)
            wbb_m = []
            for m in range(8):
                wt = res.tile([128, 24, 256], bfl, tag=f"wbb{m}")
                eng = nc.sync if m % 2 == 0 else nc.gpsimd
                eng.dma_start(wt[:], wbb_src[:, :, 256 * m:256 * (m + 1)])
                wbb_m.append(wt)
            wt1 = res.tile([128, 6], bfl)
            DMA(wt1[:], p_wt1[:])
            wt2 = res.tile([128, 6], bfl)
            DMA(wt2[:], p_wt2[:])
            wt3 = res.tile([128, 9], bfl)
            DMA(wt3[:], p_wt3[:])
            wd2 = res.tile([128, 9, 128], bfl)
            DMA(wd2[:], p_wd2.rearrange("ky kx p m -> p (ky kx) m"))
            wd3 = res.tile([128, 9, 128], bfl)
            DMA(wd3[:], p_wd3.rearrange("ky kx p m -> p (ky kx) m"))

            # image, [y-chunks] layout for crop matmuls (lhsT)
            img_yx = res.tile([128, 3, 4, 448], bfl)
            nc.gpsimd.dma_start(img_yx[0:112, :, :, :],
                x_in.rearrange("c (yc p) x -> p c yc x", p=112))

            # ---------------- main backbone conv ----------------
            f_sb = res.tile([128, 16, 196], bfl)
            feat = res.tile([128, 16], f32)
            for m in range(16):
                pm = ps_big.tile([128, 196], f32, tag="big")
                for kc in range(24):
                    nc.tensor.matmul(pm[:], wbb_m[m // 2][:, kc, 128 * (m % 2):128 * (m % 2 + 1)],
                                     im2col[:, kc, :],
                                     start=(kc == 0), stop=(kc == 23))
                nc.scalar.activation(f_sb[:, m, :], pm[:], Act.Relu,
                                     accum_out=feat[:, m:m + 1])

            # ---------------- proposal net ----------------
            fpad = res.tile([128, 16, 16, 16], bfl)
            nc.vector.memset(fpad[:], 0.0)
            nc.vector.tensor_copy(
                fpad[:, :, 1:15, 1:15],
                f_sb.rearrange("p m (y x) -> p m y x", y=14))
            d1ps = ps_big.tile([128, 196], f32, tag="big")
            for kc in range(16):
                wchunk = wstr.tile([128, 9, 128], bfl, tag="wd1")
                eng = nc.sync if kc % 2 == 0 else nc.gpsimd
                eng.dma_start(wchunk[:], p_wd1[kc].rearrange("ky kx p m -> p (ky kx) m"))
                for t9 in range(9):
                    ky, kx = divmod(t9, 3)
                    nc.tensor.matmul(
                        d1ps[:],
                        wchunk[:, t9, :],
                        fpad[:, kc, ky:ky + 14, kx:kx + 14],
                        start=(kc == 0 and t9 == 0), stop=(kc == 15 and t9 == 8))
            d1pad = res.tile([128, 16, 16], bfl)
            nc.vector.memset(d1pad[:], 0.0)
            nc.scalar.activation(
                d1pad[:, 1:15, 1:15],
                d1ps.rearrange("p (y x) -> p y x", y=14)[:],
                Act.Relu)

            t1ps = ps_sml.tile([6, 196], f32, tag="sml")
            nc.tensor.matmul(t1ps[:], wt1[:],
                             d1pad[:, 1:15, 1:15], start=True, stop=True)
            t1sb = work.tile([6, 196], f32, tag="tsb")
            nc.vector.tensor_scalar_add(t1sb[:], t1ps[:], bts[0:6, :])

            d2ps = ps_sml.tile([128, 49], f32, tag="sml")
            for t9 in range(9):
                ky, kx = divmod(t9, 3)
                nc.tensor.matmul(
                    d2ps[:], wd2[:, t9, :],
                    d1pad[:, ky:ky + 14:2, kx:kx + 14:2],
                    start=(t9 == 0), stop=(t9 == 8))
            d2pad = res.tile([128, 9, 9], bfl)
            nc.vector.memset(d2pad[:], 0.0)
            nc.scalar.activation(d2pad[:, 1:8, 1:8],
                                 d2ps.rearrange("p (y x) -> p y x", y=7)[:],
                                 Act.Relu)
            t2ps = ps_sml.tile([6, 49], f32, tag="sml")
            nc.tensor.matmul(t2ps[:], wt2[:], d2pad[:, 1:8, 1:8],
                             start=True, stop=True)
            t2sb = work.tile([6, 49], f32, tag="tsb")
            nc.vector.tensor_scalar_add(t2sb[:], t2ps[:], bts[32:38, :])

            d3ps = ps_sml.tile([128, 16], f32, tag="sml")
            for t9 in range(9):
                ky, kx = divmod(t9, 3)
                nc.tensor.matmul(
                    d3ps[:], wd3[:, t9, :],
                    d2pad[:, ky:ky + 7:2, kx:kx + 7:2],
                    start=(t9 == 0), stop=(t9 == 8))
            d3sb = work.tile([128, 16], bfl, tag="d3sb")
            nc.scalar.activation(d3sb[:], d3ps[:], Act.Relu)
            t3ps = ps_sml.tile([9, 16], f32, tag="sml")
            nc.tensor.matmul(t3ps[:], wt3[:], d3sb[:], start=True, stop=True)
            t3sb = work.tile([9, 16], f32, tag="tsb")
            nc.vector.tensor_scalar_add(t3sb[:], t3ps[:], bts[64:73, :])

            # assemble scores through DRAM bounce -> [128,13]
            sc_d = dpool.tile([1664], f32)
            DMA(sc_d[0:1176].rearrange("(a n) -> a n", a=6), t1sb[:])
            DMA(sc_d[1176:1470].rearrange("(a n) -> a n", a=6), t2sb[:])
            DMA(sc_d[1470:1614].rearrange("(a n) -> a n", a=9), t3sb[:])
            padt = work.tile([1, 50], f32, tag="padt")
            nc.vector.memset(padt[:], NEG)
            DMA(sc_d[1614:1664].rearrange("(a n) -> a n", a=1), padt[:])
            scores = nmsp.tile([128, 13], f32)
            DMA(scores[:], sc_d.rearrange("(p j) -> p j", p=128))

            # ---------------- NMS (4 rounds) ----------------
            neginf2d = nmsp.tile([128, 13], f32)
            nc.vector.memset(neginf2d[:], NEG)
            topidx = nmsp.tile([1, 4], f32)
            topprob = nmsp.tile([1, 4], f32)
            boxes = nmsp.tile([1, 16], f32)
            scratch = nmsp.tile([128, 13], f32)
            iy_t = nmsp.tile([128, 13], f32)
            ix_t = nmsp.tile([128, 13], f32)
            un_t = nmsp.tile([128, 13], f32)
            supp = nmsp.tile([128, 13], u8)
            csum = nmsp.tile([128, 5], f32)

            ty_all = nmsp.tile([1, 896], f32)
            tx_all = nmsp.tile([1, 896], f32)
            sm = nmsp.tile([1, 8], f32)
            ayt = res.tile([128, 4, 896], bfl)
            axt = res.tile([128, 4, 896], bfl)
            wtmp32 = nmsp.tile([128, 896], f32)
            wtmp = nmsp.tile([128, 896], bfl)
            S2 = dpool.tile([4, 3, 224, 32, 7], bfl)  # [t, c, i, dx, pj]
            for t in range(TOP_N):
                pm8 = nmsp.tile([128, 8], f32, tag="pm8")
                nc.vector.max(pm8[:], scores[:])
                fi8 = nmsp.tile([128, 8], u32, tag="fi8")
                nc.vector.max_index(fi8[:], pm8[:], scores[:])
                fjf = nmsp.tile([128, 1], f32, tag="fjf")
                nc.vector.tensor_copy(fjf[:], fi8[:, 0:1])
                trpsA = ps_sml.tile([1, 128], f32, tag="sml")
                nc.tensor.transpose(trpsA[:], pm8[:, 0:1], ident[:])
                trowA = nmsp.tile([1, 128], f32, tag="trowA")
                nc.vector.tensor_copy(trowA[:], trpsA[:])
                trpsB = ps_sml.tile([1, 128], f32, tag="sml")
                nc.tensor.transpose(trpsB[:], fjf[:], ident[:])
                trowB = nmsp.tile([1, 128], f32, tag="trowB")
                nc.vector.tensor_copy(trowB[:], trpsB[:])
                g8 = nmsp.tile([1, 8], f32, tag="g8")
                nc.vector.max(g8[:], trowA[:])
                nc.vector.tensor_copy(topprob[:, t:t + 1], g8[:, 0:1])
                gi8 = nmsp.tile([1, 8], u32, tag="gi8")
                nc.vector.max_index(gi8[:], g8[:], trowA[:])
                pstar = nmsp.tile([1, 2], f32, tag="pstar")
                nc.vector.tensor_copy(pstar[:, 0:1], gi8[:, 0:1])
                # jstar = jrow[p*]
                mrow = nmsp.tile([1, 128], f32, tag="mrow")
                nc.vector.scalar_tensor_tensor(
                    mrow[:], iota128r, pstar[:, 0:1], trowB[:],
                    Alu.is_equal, Alu.mult, accum_out=pstar[:, 1:2])
                # flat = p*13 + j
                nc.vector.scalar_tensor_tensor(
                    topidx[:, t:t + 1], pstar[:, 0:1], 13.0, pstar[:, 1:2],
                    Alu.mult, Alu.add)
                # broadcast flat to [128,1]
                fb_ps = ps_sml.tile([128, 1], f32, tag="sml")
                nc.tensor.matmul(fb_ps[:], ones_row, topidx[:, t:t + 1],
                                 start=True, stop=True)
                flatb = nmsp.tile([128, 1], f32, tag="flatb")
                nc.vector.tensor_copy(flatb[:], fb_ps[:])
                # gather 5 coords of picked anchor
                for k, cv in enumerate((y0a, x0a, y1a, x1a, area_a)):
                    nc.vector.scalar_tensor_tensor(
                        scratch[:], iota2d, flatb[:], cv,
                        Alu.is_equal, Alu.mult, accum_out=csum[:, k:k + 1])
                s5ps = ps_sml.tile([1, 5], f32, tag="sml")
                nc.tensor.matmul(s5ps[:], ones_col, csum[:], start=True, stop=True)
                s5 = nmsp.tile([1, 5], f32, tag="s5")
                nc.vector.tensor_copy(s5[:], s5ps[:])
                nc.vector.tensor_copy(boxes[:, 4 * t:4 * t + 4], s5[:, 0:4])
                # ---- part t crop (overlaps later NMS rounds) ----
                y0b = boxes[:, 4 * t:4 * t + 1]
                x0b = boxes[:, 4 * t + 1:4 * t + 2]
                y1b = boxes[:, 4 * t + 2:4 * t + 3]
                x1b = boxes[:, 4 * t + 3:4 * t + 4]
                nc.vector.scalar_tensor_tensor(
                    sm[:, 2 * t:2 * t + 1], y1b, -1.0, y0b, Alu.add, Alu.subtract)
                nc.vector.tensor_scalar_mul(
                    sm[:, 2 * t:2 * t + 1], sm[:, 2 * t:2 * t + 1], 1.0 / 223.0)
                nc.vector.tensor_scalar(
                    ty_all[:, 224 * t:224 * (t + 1)], iota224,
                    sm[:, 2 * t:2 * t + 1], y0b, Alu.mult, Alu.add)
                nc.vector.scalar_tensor_tensor(
                    sm[:, 2 * t + 1:2 * t + 2], x1b, -1.0, x0b, Alu.add, Alu.subtract)
                nc.vector.tensor_scalar_mul(
                    sm[:, 2 * t + 1:2 * t + 2], sm[:, 2 * t + 1:2 * t + 2], 1.0 / 223.0)
                nc.vector.tensor_scalar(
                    tx_all[:, 224 * t:224 * (t + 1)], iota224,
                    sm[:, 2 * t + 1:2 * t + 2], x0b, Alu.mult, Alu.add)
                bpsy = ps_sml.tile([128, 224], f32, tag="sml")
                nc.tensor.matmul(bpsy[:], ones_row,
                                 ty_all[:, 224 * t:224 * (t + 1)], start=True, stop=True)
                tyb_t = nmsp.tile([128, 224], f32, tag="tybt")
                nc.vector.tensor_copy(tyb_t[:], bpsy[:])
                bpsx = ps_sml.tile([128, 224], f32, tag="sml")
                nc.tensor.matmul(bpsx[:], ones_row,
                                 tx_all[:, 224 * t:224 * (t + 1)], start=True, stop=True)
                txb_t = nmsp.tile([128, 224], f32, tag="txbt")
                nc.vector.tensor_copy(txb_t[:], bpsx[:])
                for yc in range(4):
                    nc.vector.tensor_scalar_sub(wtmp32[:, 0:224], tyb_t[:],
                                                cst[:, 80 + yc:81 + yc])
                    nc.scalar.activation(wtmp[:, 0:224], wtmp32[:, 0:224], Act.Abs)
                    nc.scalar.activation(ayt[:, yc, 224 * t:224 * (t + 1)],
                                         wtmp[:, 0:224], Act.Relu,
                                         bias=ones_col, scale=-1.0)
                    nc.vector.tensor_scalar_sub(wtmp32[:, 0:224], txb_t[:],
                                                cst[:, 80 + yc:81 + yc])
                    nc.scalar.activation(wtmp[:, 0:224], wtmp32[:, 0:224], Act.Abs)
                    nc.scalar.activation(axt[:, yc, 224 * t:224 * (t + 1)],
                                         wtmp[:, 0:224], Act.Relu,
                                         bias=ones_col, scale=-1.0)
                for c in range(3):
                    T_sb = work.tile([128, 4, 224], bfl, tag="Tsb")
                    for xc in range(4):
                        tp = ps_big.tile([128, 224], f32, tag="big")
                        for yc in range(4):
                            nc.tensor.matmul(
                                tp[0:112, :],
                                img_yx[0:112, c, yc, xc * 112:(xc + 1) * 112],
                                ayt[0:112, yc, 224 * t:224 * (t + 1)],
                                start=(yc == 0), stop=(yc == 3))
                        nc.vector.tensor_copy(T_sb[0:112, xc, :], tp[0:112, :])
                    for ic in range(2):
                        pp = ps_big.tile([128, 224], f32, tag="big")
                        for xc in range(4):
                            nc.tensor.matmul(
                                pp[0:112, :],
                                T_sb[0:112, xc, 112 * ic:112 * (ic + 1)],
                                axt[0:112, xc, 224 * t:224 * (t + 1)]
                                .rearrange("p (pj dx) -> p dx pj", dx=32),
                                start=(xc == 0), stop=(xc == 3))
                        pp_sb = work.tile([128, 224], bfl, tag="ppsb")
                        nc.vector.tensor_copy(pp_sb[0:112, :], pp[0:112, :])
                        eng = nc.gpsimd if (t % 2 == 0) else nc.sync
                        eng.dma_start(
                            S2[t, c, 112 * ic:112 * (ic + 1), :, :],
                            pp_sb[0:112, :].rearrange("i (dx pj) -> i dx pj", dx=32))

                s5b_ps = ps_sml.tile([128, 5], f32, tag="sml")
                nc.tensor.matmul(s5b_ps[:], ones_row, s5[:], start=True, stop=True)
                s5b = nmsp.tile([128, 5], f32, tag="s5b")
                nc.vector.tensor_copy(s5b[:], s5b_ps[:])
                # IoU row and suppression
                nc.vector.tensor_scalar_max(scratch[:], y0a, s5b[:, 0:1])
                nc.vector.scalar_tensor_tensor(
                    iy_t[:], y1a, s5b[:, 2:3], scratch[:], Alu.min, Alu.subtract)
                nc.vector.tensor_scalar_max(iy_t[:], iy_t[:], 0.0)
                nc.vector.tensor_scalar_max(scratch[:], x0a, s5b[:, 1:2])
                nc.vector.scalar_tensor_tensor(
                    ix_t[:], x1a, s5b[:, 3:4], scratch[:], Alu.min, Alu.subtract)
                nc.vector.tensor_scalar_max(ix_t[:], ix_t[:], 0.0)
                nc.vector.tensor_tensor(ix_t[:], iy_t[:], ix_t[:], Alu.mult)  # inter
                nc.vector.scalar_tensor_tensor(
                    un_t[:], area_a, s5b[:, 4:5], ix_t[:], Alu.add, Alu.subtract)
                # suppress where 0.25*union < inter
                nc.vector.scalar_tensor_tensor(
                    supp[:], un_t[:], 0.25, ix_t[:], Alu.mult, Alu.is_lt)
                nc.vector.copy_predicated(scores[:], supp[:], neginf2d[:])

            DMA(o_tidx[:], topidx[:])
            DMA(o_tprob[:], topprob[:])

            # part im2col load [128, 24, 196] (n = t*49 + pi*7 + pj)
            rhs_p = res.tile([128, 24, 196], bfl, tag="imcol")
            for t in range(TOP_N):
                for c in range(3):
                    src_tc = S2[t, c].rearrange(
                        "(pi dyh dyl) dx pj -> dyh (dyl dx) pi pj",
                        dyh=8, dyl=4)
                    for dyh in range(8):
                        eng = nc.gpsimd if (dyh % 2 == 0) else nc.sync
                        eng.dma_start(
                            rhs_p[:, c * 8 + dyh, t * 49:(t + 1) * 49]
                            .rearrange("p (pi pj) -> p pi pj", pi=7),
                            src_tc[dyh])

            # ---------------- part backbone conv ----------------
            pf = res.tile([128, 16, 4], f32)
            po_sb = work.tile([128, 196], bfl, tag="posb")
            for m in range(16):
                pm = ps_big.tile([128, 196], f32, tag="big")
                for kc in range(24):
                    nc.tensor.matmul(pm[:], wbb_m[m // 2][:, kc, 128 * (m % 2):128 * (m % 2 + 1)],
                                     rhs_p[:, kc, :],
                                     start=(kc == 0), stop=(kc == 23))
                for t in range(TOP_N):
                    nc.scalar.activation(po_sb[:, 49 * t:49 * (t + 1)],
                                         pm[:, 49 * t:49 * (t + 1)], Act.Relu,
                                         accum_out=pf[:, m, t:t + 1])

            # ---------------- heads ----------------
            featb = res.tile([128, 16], bfl)
            nc.vector.tensor_copy(featb[:], feat[:])
            pfb = res.tile([128, 16, 4], bfl)
            nc.vector.tensor_copy(pfb[:], pf[:])

            rawps = ps_sml.tile([1, 200], f32, tag="sml")
            for m4 in range(4):
                wc = whead.tile([128, 4, 200], bfl, tag="wcat")
                DMA(wc[:], p_wfc.rearrange("(mc p) n -> p mc n", p=128)
                    [:, 4 * m4:4 * (m4 + 1), :])
                for i in range(4):
                    m = 4 * m4 + i
                    nc.tensor.matmul(rawps[:], featb[:, m:m + 1], wc[:, i, :],
                                     start=(m == 0), stop=(m == 15))
            rawsb = work.tile([1, 200], f32, tag="hdsb")
            nc.vector.tensor_copy(rawsb[:], rawps[:])
            DMA(o_raw[:], rawsb[:])

            plps = ps_sml.tile([4, 200], f32, tag="sml")
            for m4 in range(4):
                wc = whead.tile([128, 4, 200], bfl, tag="wcat")
                DMA(wc[:], p_wpart.rearrange("(mc p) n -> p mc n", p=128)
                    [:, 4 * m4:4 * (m4 + 1), :])
                for i in range(4):
                    m = 4 * m4 + i
                    nc.tensor.matmul(plps[:], pfb[:, m, :], wc[:, i, :],
                                     start=(m == 0), stop=(m == 15))
            plsb = work.tile([4, 200], f32, tag="hdsb")
            nc.vector.tensor_copy(plsb[:], plps[:])
            DMA(o_plog[:], plsb[:])

            catps = ps_sml.tile([1, 200], f32, tag="sml")
            for k4 in range(20):
                wc = whead.tile([128, 4, 200], bfl, tag="wcat")
                eng = nc.sync if k4 % 2 == 0 else nc.gpsimd
                eng.dma_start(wc[:], p_wcat.rearrange("(kc p) n -> p kc n", p=128)
                              [:, 4 * k4:4 * (k4 + 1), :])
                for i in range(4):
                    kc = 4 * k4 + i
                    if kc < 64:
                        t, m = divmod(kc, 16)
                        lhs = pfb[:, m, t:t + 1]
                    else:
                        m = kc - 64
                        lhs = featb[:, m:m + 1]
                    nc.tensor.matmul(catps[:], lhs, wc[:, i, :],
                                     start=(kc == 0), stop=(kc == 79))
            catsb = work.tile([1, 200], f32, tag="hdsb")
            nc.vector.tensor_copy(catsb[:], catps[:])
            DMA(o_cat[:], catsb[:])

    nc.compile()
    return nc


_NC = None
_RUN_KW = {}


def _get_nc():
    global _NC
    if _NC is None:
        _NC = build_kernel()
    return _NC


def kernel(**inputs):
    inputs = {k: np.asarray(v) for k, v in inputs.items()}
    nc = _get_nc()
    from concourse.bass_utils import run_bass_kernel_spmd

    W = _prep_weights(inputs)
    cst, crowc, ident = _host_consts()
    base = dict(W)
    base["cst"] = cst
    base["crow"] = crowc
    base["ident"] = ident
    in_maps = []
    for b in range(BATCH):
        m = dict(base)
        xb = inputs["x"][b]
        m["x"] = np.ascontiguousarray(xb).astype(bf16)
        m["xim"] = np.ascontiguousarray(
            xb.reshape(3, 14, 32, 14, 32).transpose(0, 2, 4, 1, 3)
            .reshape(3072, 196)).astype(bf16)
        in_maps.append(m)
    res = run_bass_kernel_spmd(nc, in_maps, list(range(BATCH)),
                               **_RUN_KW)
    r = res.results
    kernel.last_result = res
    raw = np.stack([r[b]["o_raw"][0] for b in range(BATCH)]).astype(np.float32)
    cat = np.stack([r[b]["o_cat"][0] for b in range(BATCH)]).astype(np.float32)
    plog = np.stack([r[b]["o_plog"] for b in range(BATCH)]).astype(np.float32)
    tidx = np.stack([r[b]["o_tidx"][0] for b in range(BATCH)])
    tprob = np.stack([r[b]["o_tprob"][0] for b in range(BATCH)]).astype(np.float32)
    tidx = np.rint(tidx).astype(np.int32)
    return raw, cat, plog, tidx, tprob


if __name__ == "__main__":
    d = np.load("/tmp/real_inputs.npz")
    out = kernel(**{k: d[k] for k in d.files})
    for o in out:
        print(o.shape, o.dtype)
